# revision 23
# baseline (speedup 1.0000x reference)
"""Trainium2 Bass kernel for nn_AttentionBlock (B=4, N=2048, IN_C=256, H=8,
HEAD_C=32, EXPAND=1024), distributed over 8 NeuronCores.

Sharding: data-parallel over query rows. Core c handles batch c//2; its xb
input is the batch ROTATED so the core's 1024 query rows are always rows
0..1023 (softmax is invariant to key permutation, so one compiled graph
serves all cores). Each core computes K/V for its full batch locally, so no
collectives: the head-merge (out @ Wm) is folded into the value projection
(Wvm_h = Wv_h @ Wm_h, host-precomputed) and summed over heads locally.
LayerNorm gains/biases are folded into the following matmul weights/biases on
the host, so in-kernel LN is just stats + one fused normalize.

Layout: channel-major activations feed TensorE directly. S^T tiles
[keys, q] -> exp on ScalarE (no max-subtraction; |S|~N(0,1)) -> PV matmul
accumulates O = exp(S^T).T @ [v | 1] in PSUM; the ones-column yields softmax
row-sums, so normalization is a reciprocal + fused scale-add at eviction.
The S matmul uses tight-packed kT (4 heads x 32ch = 128 partitions) as the
stationary operand -- full-128 weights keep FWL on so LDWEIGHTS stays hidden
-- against zero-padded per-head qT as the moving operand. All matmuls bf16
with fp32 PSUM accumulation.
"""
import numpy as np
import ml_dtypes

import concourse.bass as bass
import concourse.bacc as bacc
import concourse.tile as tile
from concourse import mybir
from concourse.bass_utils import run_bass_kernel_spmd
from concourse.masks import make_identity


def _ensure_ntff_hook():
    """Register antenv.axon_hooks if the image's antenv lacks it, so
    BASS_TRACE=1 profiling works (ctypes into libaxon_pjrt.so). No-op on
    any failure — tracing degrades gracefully in bass_utils."""
    import sys
    try:
        import antenv.axon_hooks  # noqa: F401
        return
    except ImportError:
        pass
    try:
        import contextlib
        import ctypes
        import types

        so_path = "/opt/axon/libaxon_pjrt.so"
        lib = ctypes.CDLL(so_path)
        if not hasattr(lib, "axon_start_nrt_profile"):
            return
        lib.axon_start_nrt_profile.argtypes = [
            ctypes.POINTER(ctypes.c_int64), ctypes.c_size_t]
        lib.axon_start_nrt_profile.restype = ctypes.c_int64
        lib.axon_stop_nrt_profile.argtypes = [ctypes.c_char_p]
        lib.axon_stop_nrt_profile.restype = ctypes.c_int64

        @contextlib.contextmanager
        def _hook(output_dir, device_ids):
            import jax
            jax.devices()
            if device_ids:
                ids = (ctypes.c_int64 * len(device_ids))(*device_ids)
                rc = lib.axon_start_nrt_profile(ids, len(device_ids))
            else:
                rc = lib.axon_start_nrt_profile(None, 0)
            if rc != 0:
                raise RuntimeError(f"axon_start_nrt_profile rc={rc}")
            try:
                yield
            finally:
                n = lib.axon_stop_nrt_profile(str(output_dir).encode())
                if n < 0:
                    raise RuntimeError(f"axon_stop_nrt_profile rc={n}")

        mod = types.ModuleType("antenv.axon_hooks")
        mod.get_axon_ntff_profile_hook = lambda: _hook
        mod.set_axon_ntff_profile_hook = lambda h: None
        sys.modules["antenv.axon_hooks"] = mod
    except Exception:
        pass


_ensure_ntff_hook()

F32 = mybir.dt.float32
BF16 = mybir.dt.bfloat16
AF = mybir.ActivationFunctionType
ALU = mybir.AluOpType
P = 128

B, N, IN_C, HEAD_C, H, EXPAND = 4, 2048, 256, 32, 8, 1024
NQ = 1024            # query rows per core
NT_B = N // P        # 16 token tiles per batch
NT_Q = NQ // P       # 8 query tiles per core
LN_EPS = 1e-5
VE_W = 2 * IN_C + 2  # 514: per key-tile [v_h0(256) | 1 | v_h1(256) | 1]

_NC = None  # cached compiled graph


def _build(do_compile=True):
    nc = bacc.Bacc("TRN2", target_bir_lowering=False, debug=False, num_devices=8)

    xb_e = nc.dram_tensor("xb", [N, IN_C], F32, kind="ExternalInput")
    # q/k weights, head-padded: block b holds head 2b at col 0 and head 2b+1
    # at col 64 (32 cols each, zeros elsewhere) so S operands sit at PE row
    # bands 0/64 (band 96 is unusable: PE quadrant-3 HW bug).
    wq_e = nc.dram_tensor("wq", [IN_C, 4 * P], BF16, kind="ExternalInput")
    wk_e = nc.dram_tensor("wk", [IN_C, 4 * P], BF16, kind="ExternalInput")
    wvm_e = nc.dram_tensor("wvm", [IN_C, H * IN_C], BF16, kind="ExternalInput")
    w1_e = nc.dram_tensor("w1", [IN_C, EXPAND], BF16, kind="ExternalInput")
    w2_e = nc.dram_tensor("w2", [EXPAND, IN_C], BF16, kind="ExternalInput")
    battn_e = nc.dram_tensor("battn", [IN_C], F32, kind="ExternalInput")
    b2_e = nc.dram_tensor("b2", [IN_C], F32, kind="ExternalInput")
    bq_e = nc.dram_tensor("bq", [P, 4], F32, kind="ExternalInput")
    bk_e = nc.dram_tensor("bk", [P, 4], F32, kind="ExternalInput")
    b1_e = nc.dram_tensor("b1", [P, EXPAND // P], F32, kind="ExternalInput")
    # one output param per 128-row tile: keeps each device->host transfer at
    # 128KB (larger single transfers can wedge the axon stdio relay)
    out_es = [nc.dram_tensor(f"out{i}", [P, IN_C], F32, kind="ExternalOutput")
              for i in range(NT_Q)]

    from contextlib import ExitStack
    with tile.TileContext(nc) as tc, ExitStack() as ctx:
        const = ctx.enter_context(tc.tile_pool(name="const", bufs=1))
        persist = ctx.enter_context(tc.tile_pool(name="persist", bufs=1))
        lnt = ctx.enter_context(tc.tile_pool(name="lnt", bufs=3))
        stats_p = ctx.enter_context(tc.tile_pool(name="stats", bufs=4))
        exps_p = ctx.enter_context(tc.tile_pool(name="exps", bufs=4))
        on_p = ctx.enter_context(tc.tile_pool(name="on", bufs=3))
        st_ps = ctx.enter_context(tc.tile_pool(name="st_ps", bufs=2, space="PSUM"))
        o_ps = ctx.enter_context(tc.tile_pool(name="o_ps", bufs=2, space="PSUM"))
        mm_ps = ctx.enter_context(tc.tile_pool(name="mm_ps", bufs=2, space="PSUM"))

        # ---- constants ----
        # x DMAs first: LN1 is the critical path at startup
        xbig = const.tile([P, NT_B * IN_C], F32, tag="xbig", name="xbig")
        xb_src = xb_e.ap().rearrange("(t p) c -> p t c", p=P)
        xbig_v = xbig[:].rearrange("p (t c) -> p t c", t=NT_B)
        for d in range(8):
            nc.sync.dma_start(out=xbig_v[:, d * 2:(d + 1) * 2, :],
                              in_=xb_src[:, d * 2:(d + 1) * 2, :])

        def rep256(name, ext):
            t = const.tile([P, IN_C], F32, tag=name, name=name)
            nc.sync.dma_start(
                out=t[:], in_=ext.ap()[None, :].broadcast_to([P, IN_C]))
            return t

        battn_r = rep256("battn", battn_e)
        b2_r = rep256("b2", b2_e)
        bq_t = const.tile([P, 4], F32, tag="bq", name="bq")
        nc.sync.dma_start(out=bq_t[:], in_=bq_e[:])
        bk_t = const.tile([P, 4], F32, tag="bk", name="bk")
        nc.sync.dma_start(out=bk_t[:], in_=bk_e[:])
        b1_t = const.tile([P, EXPAND // P], F32, tag="b1", name="b1")
        nc.sync.dma_start(out=b1_t[:], in_=b1_e[:])
        eps_t = const.tile([P, 1], F32, tag="eps", name="eps")
        nc.vector.memset(eps_t[:], LN_EPS)
        ident = const.tile([P, P], BF16, tag="ident", name="ident")
        make_identity(nc, ident[:])

        # weights: K-chunk tensors
        wq_s = [const.tile([P, 4 * P], BF16, tag=f"wq{k}", name=f"wq{k}") for k in range(2)]
        wk_s = [const.tile([P, 4 * P], BF16, tag=f"wk{k}", name=f"wk{k}") for k in range(2)]
        wvm_s = [const.tile([P, H * IN_C], BF16, tag=f"wvm{k}", name=f"wvm{k}") for k in range(2)]
        w1_s = [const.tile([P, EXPAND], BF16, tag=f"w1{k}", name=f"w1{k}") for k in range(2)]
        for k in range(2):
            nc.sync.dma_start(out=wq_s[k][:], in_=wq_e[k * P:(k + 1) * P, :])
            nc.sync.dma_start(out=wk_s[k][:], in_=wk_e[k * P:(k + 1) * P, :])
            nc.sync.dma_start(out=wvm_s[k][:], in_=wvm_e[k * P:(k + 1) * P, :])
            nc.sync.dma_start(out=w1_s[k][:], in_=w1_e[k * P:(k + 1) * P, :])
        w2_s = const.tile([P, EXPAND // P * IN_C], BF16, tag="w2", name="w2")  # [128, 8*256]
        nc.sync.dma_start(
            out=w2_s[:].rearrange("p (e c) -> p e c", e=EXPAND // P),
            in_=w2_e.ap().rearrange("(e p) c -> p e c", p=P),
        )

        # ---- persistent activations ----
        ln_b_T = [persist.tile([P, N], BF16, tag=f"lnbT{k}", name=f"lnbT{k}") for k in range(2)]
        qT = [persist.tile([P, NQ], BF16, tag=f"qT{g}", name=f"qT{g}") for g in range(4)]
        kT = [persist.tile([P, N], BF16, tag=f"kT{g}", name=f"kT{g}") for g in range(4)]
        x2acc = [persist.tile([P, IN_C], F32, tag=f"x2acc{i}", name=f"x2acc{i}") for i in range(NT_Q)]
        x2T = [persist.tile([P, NQ], BF16, tag=f"x2T{k}", name=f"x2T{k}") for k in range(2)]
        hT = persist.tile([P, EXPAND // P * NQ], BF16, tag="hT", name="hT")  # [128, 8*1024]
        ve = [persist.tile([P, NT_B * VE_W], BF16, tag=f"ve{i}", name=f"ve{i}")
              for i in range(2)]
        # ones columns at 256 + 257k, constant for the whole kernel
        for i in range(2):
            nc.gpsimd.memset(ve[i][:, IN_C::IN_C + 1], 1.0)

        # ---- LayerNorm: stats on Vector, sqrt + normalize on Scalar
        def ln_tile_sb(x_ap):
            stats = stats_p.tile([P, 6], F32, tag="stats", name="stats")
            mv = stats_p.tile([P, 2], F32, tag="mv", name="mv")
            nc.vector.bn_stats(out=stats[:], in_=x_ap)
            nc.vector.bn_aggr(out=mv[:], in_=stats[:])
            sd = stats_p.tile([P, 1], F32, tag="sd", name="sd")
            nc.scalar.activation(out=sd[:], in_=mv[:, 1:2], func=AF.Sqrt,
                                 bias=eps_t[:])
            nc.vector.reciprocal(out=sd[:], in_=sd[:])
            nmsd = stats_p.tile([P, 1], F32, tag="nmsd", name="nmsd")
            nc.vector.tensor_scalar(
                out=nmsd[:], in0=mv[:, 0:1], scalar1=sd[:], scalar2=-1.0,
                op0=ALU.mult, op1=ALU.mult)
            xnb = lnt.tile([P, IN_C], BF16, tag="ln_xnb", name="ln_xnb")
            nc.scalar.activation(out=xnb[:], in_=x_ap, func=AF.Identity,
                                 scale=sd[:], bias=nmsd[:])
            return xnb

        def transpose_into(dstlist, col, xnb):
            for k in range(2):
                tp = mm_ps.tile([P, P], BF16, tag="proj", name="proj")
                nc.tensor.transpose(tp[:], xnb[:, k * P:(k + 1) * P], ident[:])
                nc.vector.tensor_copy(out=dstlist[k][:, col:col + P], in_=tp[:])

        # ---- Phase A: LN1 over the 16 batch tiles (queries are tiles 0-7)
        for t in range(NT_B):
            xnb = ln_tile_sb(xbig[:, t * IN_C:(t + 1) * IN_C])
            transpose_into(ln_b_T, t * P, xnb)

        # ---- init x2acc = xq + battn (on GpSimd) ----
        for qi in range(NT_Q):
            nc.gpsimd.tensor_add(out=x2acc[qi][:],
                                 in0=xbig[:, qi * IN_C:(qi + 1) * IN_C],
                                 in1=battn_r[:])

        # ---- Phase B: q/k projections (channel-major, 2 heads/block);
        # k before q within a block so S(U0) can start as early as possible
        for g in range(4):
            for nn in range(N // 512):
                ps = mm_ps.tile([P, 512], F32, tag="proj", name="proj")
                for k in range(2):
                    nc.tensor.matmul(
                        ps[:], wk_s[k][:, g * P:(g + 1) * P],
                        ln_b_T[k][:, nn * 512:(nn + 1) * 512],
                        start=(k == 0), stop=(k == 1))
                nc.vector.tensor_scalar_add(
                    out=kT[g][:, nn * 512:(nn + 1) * 512], in0=ps[:],
                    scalar1=bk_t[:, g:g + 1])
            for nn in range(NQ // 512):
                ps = mm_ps.tile([P, 512], F32, tag="proj", name="proj")
                for k in range(2):
                    nc.tensor.matmul(
                        ps[:], wq_s[k][:, g * P:(g + 1) * P],
                        ln_b_T[k][:, nn * 512:(nn + 1) * 512],
                        start=(k == 0), stop=(k == 1))
                nc.vector.tensor_scalar_add(
                    out=qT[g][:, nn * 512:(nn + 1) * 512], in0=ps[:],
                    scalar1=bq_t[:, g:g + 1])

        # ---- Phase C: attention, software-pipelined over 8 (pair, qb) units.
        # Unit i runs its S matmuls (2-band concurrent: the pair's heads sit
        # at different 32-partition bands of the PE array) + exp, exp-paced on
        # ScalarE; unit i-1's PV chunks and later pairs' ve-builds fill the
        # TensorE stalls in between.
        units = [(p, qb) for p in range(4) for qb in range(2)]

        def ve_chunk(p, t):
            # values for heads 2p, 2p+1, key-tile t: [token, 514] with ones
            ps = mm_ps.tile([P, 512], F32, tag="proj", name="proj")
            for k in range(2):
                nc.tensor.matmul(
                    ps[:], ln_b_T[k][:, t * P:(t + 1) * P],
                    wvm_s[k][:, 2 * p * IN_C:(2 * p + 2) * IN_C],
                    start=(k == 0), stop=(k == 1))
            dst = ve[p % 2][:, t * VE_W:(t + 1) * VE_W] \
                .rearrange("p (h c) -> p h c", h=2)[:, :, 0:IN_C]
            src = ps[:].rearrange("p (h c) -> p h c", h=2)
            nc.vector.tensor_copy(out=dst, in_=src)

        # step (i, kc2) -> ve chunks to interleave, scheduled so each pair's
        # ve is complete before its first PV drain and never overwrites a ve
        # buffer still being read by in-flight PV units.
        VE_SCHED = {0: [(0, 2)], 1: [(1, 2)], 3: [(2, 1)],
                    4: [(2, 1, 8)], 5: [(3, 1)], 6: [(3, 1, 8)]}

        def s_group(u, ex, kc2):
            (p, qb) = u
            sts = [st_ps.tile([P, 1024], F32, tag="st", name="st")
                   for _ in range(2)]
            for j in range(2):
                kc = kc2 * 2 + j
                for h2 in range(2):
                    b = h2 * 64
                    nc.tensor.matmul(
                        sts[h2][:, j * 512:(j + 1) * 512],
                        kT[p][b:b + 32, kc * P:(kc + 1) * P],
                        qT[p][b:b + 32, qb * 512:(qb + 1) * 512],
                        start=True, stop=True)
            for h2 in range(2):
                nc.scalar.activation(
                    out=ex[h2][:, kc2 * 1024:(kc2 + 1) * 1024],
                    in_=sts[h2][:], func=AF.Exp)

        def pv_chunk(u, ex, c):
            (p, qb) = u
            h2, qt = c // 4, c % 4
            op = o_ps.tile([P, IN_C + 1], F32, tag="opv", name="opv")
            for kc in range(NT_B):
                nc.tensor.matmul(
                    op[:],
                    ex[h2][:, kc * 512 + qt * P: kc * 512 + (qt + 1) * P],
                    ve[p % 2][:, kc * VE_W + h2 * (IN_C + 1):
                              kc * VE_W + h2 * (IN_C + 1) + IN_C + 1],
                    start=(kc == 0), stop=(kc == NT_B - 1))
            rc = stats_p.tile([P, 1], F32, tag="rc", name="rc")
            nc.vector.reciprocal(out=rc[:], in_=op[:, IN_C:IN_C + 1])
            qi = qb * 4 + qt
            nc.vector.scalar_tensor_tensor(
                out=x2acc[qi][:], in0=op[:, 0:IN_C], scalar=rc[:],
                in1=x2acc[qi][:], op0=ALU.mult, op1=ALU.add)

        # ---- Phase D building blocks (interleaved into the final PV drain)
        def ln2_unit(qi):
            xnb = ln_tile_sb(x2acc[qi][:])
            transpose_into(x2T, qi * P, xnb)
            # after LN2 read, fold b2 into the residual
            nc.vector.tensor_add(out=x2acc[qi][:], in0=x2acc[qi][:],
                                 in1=b2_r[:])

        def ffn1_unit(nh, ebs):
            for eb in ebs:
                ps = mm_ps.tile([P, 512], F32, tag="proj", name="proj")
                for k in range(2):
                    nc.tensor.matmul(
                        ps[:], w1_s[k][:, eb * P:(eb + 1) * P],
                        x2T[k][:, nh * 512:(nh + 1) * 512],
                        start=(k == 0), stop=(k == 1))
                nc.scalar.activation(
                    out=hT[:, eb * NQ + nh * 512: eb * NQ + (nh + 1) * 512],
                    in_=ps[:], func=AF.Gelu, bias=b1_t[:, eb:eb + 1])

        def ffn2_unit(qi):
            op = o_ps.tile([P, IN_C + 1], F32, tag="opv", name="opv")
            for eb in range(EXPAND // P):
                nc.tensor.matmul(
                    op[:, 0:IN_C],
                    hT[:, eb * NQ + qi * P: eb * NQ + (qi + 1) * P],
                    w2_s[:, eb * IN_C:(eb + 1) * IN_C],
                    start=(eb == 0), stop=(eb == EXPAND // P - 1))
            ot = on_p.tile([P, IN_C], F32, tag="out", name="out")
            nc.vector.tensor_add(out=ot[:], in0=op[:, 0:IN_C], in1=x2acc[qi][:])
            nc.sync.dma_start(out=out_es[qi][:], in_=ot[:])

        def ln2a(qi):
            xnb = ln_tile_sb(x2acc[qi][:])
            transpose_into(x2T, qi * P, xnb)

        # FFN work for query tiles 0-3 (final once unit 6 drained) runs
        # inside unit 7's PV drain; step index -> thunks.
        def _b2adds03():
            for qi in range(4):
                nc.vector.tensor_add(out=x2acc[qi][:], in0=x2acc[qi][:],
                                     in1=b2_r[:])
        FFN_A = {
            0: [lambda: ln2a(0)], 1: [lambda: ln2a(1)],
            2: [lambda: ln2a(2)], 3: [lambda: ln2a(3), _b2adds03],
            4: [lambda: ffn1_unit(0, range(0, 4))],
            5: [lambda: ffn1_unit(0, range(4, 8))],
            6: [lambda: ffn2_unit(0), lambda: ffn2_unit(1)],
            7: [lambda: ffn2_unit(2), lambda: ffn2_unit(3)],
        }

        exps_of = {}
        for i in range(len(units) + 1):
            if i < len(units):
                exps_of[i] = [
                    exps_p.tile([P, NT_B * 512], BF16, tag="exps", name="exps")
                    for _ in range(2)]
            for kc2 in range(8):
                if i < len(units):
                    s_group(units[i], exps_of[i], kc2)
                for sched in VE_SCHED.get(i, []):
                    p_, n_ = sched[0], sched[1]
                    t0 = (sched[2] if len(sched) > 2 else 0) + kc2 * n_
                    for t in range(t0, t0 + n_):
                        ve_chunk(p_, t)
                if i > 0:
                    pv_chunk(units[i - 1], exps_of[i - 1], kc2)
                    if i == len(units):
                        for thunk in FFN_A.get(kc2, []):
                            thunk()
            if i > 0:
                del exps_of[i - 1]

        # ---- Phase D tail: query tiles 4-7 ----
        for qi in range(4, NT_Q):
            ln2a(qi)
        for qi in range(4, NT_Q):
            nc.vector.tensor_add(out=x2acc[qi][:], in0=x2acc[qi][:],
                                 in1=b2_r[:])
        ffn1_unit(1, range(EXPAND // P))
        for qi in range(4, NT_Q):
            ffn2_unit(qi)

    if do_compile:
        nc.compile()
    return nc


def _to_bf16(a):
    return np.asarray(a, dtype=np.float32).astype(ml_dtypes.bfloat16)


def kernel(x, ln1_g, ln1_b, Wqkv, bqkv, Wm, bm, ln2_g, ln2_b, W1, b1, W2, b2):
    global _NC
    x = np.asarray(x, dtype=np.float32)
    ln1_g = np.asarray(ln1_g, dtype=np.float32)
    ln1_b = np.asarray(ln1_b, dtype=np.float32)
    Wqkv = np.asarray(Wqkv, dtype=np.float32)
    bqkv = np.asarray(bqkv, dtype=np.float32)
    Wm = np.asarray(Wm, dtype=np.float32)
    bm = np.asarray(bm, dtype=np.float32)
    ln2_g = np.asarray(ln2_g, dtype=np.float32)
    ln2_b = np.asarray(ln2_b, dtype=np.float32)
    W1 = np.asarray(W1, dtype=np.float32)
    b1 = np.asarray(b1, dtype=np.float32)
    W2 = np.asarray(W2, dtype=np.float32)
    b2 = np.asarray(b2, dtype=np.float32)

    scale = HEAD_C ** -0.5
    # fold LN1 gain into the q/k/v weight rows; LN1 bias into the biases
    Wq_f = ln1_g[:, None] * Wqkv[:, :IN_C]
    Wk_f = ln1_g[:, None] * Wqkv[:, IN_C:2 * IN_C]
    Wv_f = ln1_g[:, None] * Wqkv[:, 2 * IN_C:]
    bq_f = bqkv[:IN_C] + ln1_b @ Wqkv[:, :IN_C]
    bk_f = bqkv[IN_C:2 * IN_C] + ln1_b @ Wqkv[:, IN_C:2 * IN_C]
    bv_f = bqkv[2 * IN_C:] + ln1_b @ Wqkv[:, 2 * IN_C:]

    # head-padded q/k layouts: block b = heads 2b (col 0) / 2b+1 (col 64)
    wq = np.zeros((IN_C, 4 * P), np.float32)
    wk = np.zeros((IN_C, 4 * P), np.float32)
    bq = np.zeros((P, 4), np.float32)
    bk = np.zeros((P, 4), np.float32)
    for h in range(H):
        g, off = h // 2, 64 * (h % 2)
        wq[:, g * P + off: g * P + off + HEAD_C] = \
            Wq_f[:, h * HEAD_C:(h + 1) * HEAD_C] * scale
        wk[:, g * P + off: g * P + off + HEAD_C] = \
            Wk_f[:, h * HEAD_C:(h + 1) * HEAD_C]
        bq[off:off + HEAD_C, g] = bq_f[h * HEAD_C:(h + 1) * HEAD_C] * scale
        bk[off:off + HEAD_C, g] = bk_f[h * HEAD_C:(h + 1) * HEAD_C]
    wq = _to_bf16(wq)
    wk = _to_bf16(wk)
    wvm = np.empty((IN_C, H * IN_C), np.float32)
    for h in range(H):
        Wv_h = Wv_f[:, h * IN_C:(h + 1) * IN_C]
        Wm_h = Wm[h * IN_C:(h + 1) * IN_C, :]
        wvm[:, h * IN_C:(h + 1) * IN_C] = Wv_h @ Wm_h
    wvm = _to_bf16(wvm)
    battn = (bm + bv_f @ Wm).astype(np.float32)

    # fold LN2 gain into W1 rows, LN2 bias into b1
    W1_f = ln2_g[:, None] * W1
    b1_f = b1 + ln2_b @ W1
    b1_l = np.ascontiguousarray(b1_f.reshape(EXPAND // P, P).T).astype(np.float32)

    common = dict(
        wq=wq, wk=wk, wvm=wvm, w1=_to_bf16(W1_f), w2=_to_bf16(W2),
        battn=battn, b2=b2, bq=bq, bk=bk, b1=b1_l,
    )
    in_maps = []
    for c in range(8):
        b, qh = c // 2, c % 2
        # rotate the batch so this core's query rows are rows 0..1023
        xb = x[b] if qh == 0 else np.concatenate([x[b, NQ:], x[b, :NQ]])
        in_maps.append(dict(xb=np.ascontiguousarray(xb), **common))

    global LAST_RESULT
    if _NC is None:
        _NC = _build()
    res = run_bass_kernel_spmd(_NC, in_maps, core_ids=list(range(8)))
    LAST_RESULT = res  # exposes exec_time_ns when BASS_TRACE=1 is set
    out = np.concatenate(
        [np.asarray(res.results[c][f"out{i}"])
         for c in range(8) for i in range(NT_Q)], axis=0)
    return out.reshape(B, N, IN_C)


LAST_RESULT = None


# revision 24
# speedup vs baseline: 1.0148x; 1.0148x over previous
"""Trainium2 Bass kernel for nn_AttentionBlock (B=4, N=2048, IN_C=256, H=8,
HEAD_C=32, EXPAND=1024), distributed over 8 NeuronCores.

Sharding: data-parallel over query rows. Core c handles batch c//2; its xb
input is the batch ROTATED so the core's 1024 query rows are always rows
0..1023 (softmax is invariant to key permutation, so one compiled graph
serves all cores). Each core computes K/V for its full batch locally, so no
collectives: the head-merge (out @ Wm) is folded into the value projection
(Wvm_h = Wv_h @ Wm_h, host-precomputed) and summed over heads locally.
LayerNorm gains/biases are folded into the following matmul weights/biases on
the host, so in-kernel LN is just stats + one fused normalize.

Layout: channel-major activations feed TensorE directly. S^T tiles
[keys, q] -> exp on ScalarE (no max-subtraction; |S|~N(0,1)) -> PV matmul
accumulates O = exp(S^T).T @ [v | 1] in PSUM; the ones-column yields softmax
row-sums, so normalization is a reciprocal + fused scale-add at eviction.
The S matmul uses tight-packed kT (4 heads x 32ch = 128 partitions) as the
stationary operand -- full-128 weights keep FWL on so LDWEIGHTS stays hidden
-- against zero-padded per-head qT as the moving operand. All matmuls bf16
with fp32 PSUM accumulation.
"""
import numpy as np
import ml_dtypes

import concourse.bass as bass
import concourse.bacc as bacc
import concourse.tile as tile
from concourse import mybir
from concourse.bass_utils import run_bass_kernel_spmd
from concourse.masks import make_identity


def _ensure_ntff_hook():
    """Register antenv.axon_hooks if the image's antenv lacks it, so
    BASS_TRACE=1 profiling works (ctypes into libaxon_pjrt.so). No-op on
    any failure — tracing degrades gracefully in bass_utils."""
    import sys
    try:
        import antenv.axon_hooks  # noqa: F401
        return
    except ImportError:
        pass
    try:
        import contextlib
        import ctypes
        import types

        so_path = "/opt/axon/libaxon_pjrt.so"
        lib = ctypes.CDLL(so_path)
        if not hasattr(lib, "axon_start_nrt_profile"):
            return
        lib.axon_start_nrt_profile.argtypes = [
            ctypes.POINTER(ctypes.c_int64), ctypes.c_size_t]
        lib.axon_start_nrt_profile.restype = ctypes.c_int64
        lib.axon_stop_nrt_profile.argtypes = [ctypes.c_char_p]
        lib.axon_stop_nrt_profile.restype = ctypes.c_int64

        @contextlib.contextmanager
        def _hook(output_dir, device_ids):
            import jax
            jax.devices()
            if device_ids:
                ids = (ctypes.c_int64 * len(device_ids))(*device_ids)
                rc = lib.axon_start_nrt_profile(ids, len(device_ids))
            else:
                rc = lib.axon_start_nrt_profile(None, 0)
            if rc != 0:
                raise RuntimeError(f"axon_start_nrt_profile rc={rc}")
            try:
                yield
            finally:
                n = lib.axon_stop_nrt_profile(str(output_dir).encode())
                if n < 0:
                    raise RuntimeError(f"axon_stop_nrt_profile rc={n}")

        mod = types.ModuleType("antenv.axon_hooks")
        mod.get_axon_ntff_profile_hook = lambda: _hook
        mod.set_axon_ntff_profile_hook = lambda h: None
        sys.modules["antenv.axon_hooks"] = mod
    except Exception:
        pass


_ensure_ntff_hook()

F32 = mybir.dt.float32
BF16 = mybir.dt.bfloat16
AF = mybir.ActivationFunctionType
ALU = mybir.AluOpType
P = 128

B, N, IN_C, HEAD_C, H, EXPAND = 4, 2048, 256, 32, 8, 1024
NQ = 1024            # query rows per core
NT_B = N // P        # 16 token tiles per batch
NT_Q = NQ // P       # 8 query tiles per core
LN_EPS = 1e-5
VE_W = 2 * IN_C + 2  # 514: per key-tile [v_h0(256) | 1 | v_h1(256) | 1]

_NC = None  # cached compiled graph


def _build(do_compile=True):
    nc = bacc.Bacc("TRN2", target_bir_lowering=False, debug=False, num_devices=8)

    xb_e = nc.dram_tensor("xb", [N, IN_C], F32, kind="ExternalInput")
    # q/k weights, head-padded: block b holds head 2b at col 0 and head 2b+1
    # at col 64 (32 cols each, zeros elsewhere) so S operands sit at PE row
    # bands 0/64 (band 96 is unusable: PE quadrant-3 HW bug).
    wq_e = nc.dram_tensor("wq", [IN_C, 4 * P], BF16, kind="ExternalInput")
    wk_e = nc.dram_tensor("wk", [IN_C, 4 * P], BF16, kind="ExternalInput")
    wvm_e = nc.dram_tensor("wvm", [IN_C, H * IN_C], BF16, kind="ExternalInput")
    w1_e = nc.dram_tensor("w1", [IN_C, EXPAND], BF16, kind="ExternalInput")
    w2_e = nc.dram_tensor("w2", [EXPAND, IN_C], BF16, kind="ExternalInput")
    battn_e = nc.dram_tensor("battn", [IN_C], F32, kind="ExternalInput")
    b2_e = nc.dram_tensor("b2", [IN_C], F32, kind="ExternalInput")
    bq_e = nc.dram_tensor("bq", [P, 4], F32, kind="ExternalInput")
    bk_e = nc.dram_tensor("bk", [P, 4], F32, kind="ExternalInput")
    b1_e = nc.dram_tensor("b1", [P, EXPAND // P], F32, kind="ExternalInput")
    # one output param per 128-row tile: keeps each device->host transfer at
    # 128KB (larger single transfers can wedge the axon stdio relay)
    out_es = [nc.dram_tensor(f"out{i}", [P, IN_C], F32, kind="ExternalOutput")
              for i in range(NT_Q)]

    from contextlib import ExitStack
    with tile.TileContext(nc) as tc, ExitStack() as ctx:
        const = ctx.enter_context(tc.tile_pool(name="const", bufs=1))
        persist = ctx.enter_context(tc.tile_pool(name="persist", bufs=1))
        lnt = ctx.enter_context(tc.tile_pool(name="lnt", bufs=3))
        stats_p = ctx.enter_context(tc.tile_pool(name="stats", bufs=4))
        exps_p = ctx.enter_context(tc.tile_pool(name="exps", bufs=4))
        on_p = ctx.enter_context(tc.tile_pool(name="on", bufs=3))
        st_ps = ctx.enter_context(tc.tile_pool(name="st_ps", bufs=2, space="PSUM"))
        o_ps = ctx.enter_context(tc.tile_pool(name="o_ps", bufs=2, space="PSUM"))
        mm_ps = ctx.enter_context(tc.tile_pool(name="mm_ps", bufs=2, space="PSUM"))

        # ---- constants ----
        # x DMAs first: LN1 is the critical path at startup
        xbig = const.tile([P, NT_B * IN_C], F32, tag="xbig", name="xbig")
        xb_src = xb_e.ap().rearrange("(t p) c -> p t c", p=P)
        xbig_v = xbig[:].rearrange("p (t c) -> p t c", t=NT_B)
        for d in range(8):
            nc.sync.dma_start(out=xbig_v[:, d * 2:(d + 1) * 2, :],
                              in_=xb_src[:, d * 2:(d + 1) * 2, :])

        def rep256(name, ext):
            t = const.tile([P, IN_C], F32, tag=name, name=name)
            nc.sync.dma_start(
                out=t[:], in_=ext.ap()[None, :].broadcast_to([P, IN_C]))
            return t

        battn_r = rep256("battn", battn_e)
        b2_r = rep256("b2", b2_e)
        bq_t = const.tile([P, 4], F32, tag="bq", name="bq")
        nc.sync.dma_start(out=bq_t[:], in_=bq_e[:])
        bk_t = const.tile([P, 4], F32, tag="bk", name="bk")
        nc.sync.dma_start(out=bk_t[:], in_=bk_e[:])
        b1_t = const.tile([P, EXPAND // P], F32, tag="b1", name="b1")
        nc.sync.dma_start(out=b1_t[:], in_=b1_e[:])
        eps_t = const.tile([P, 1], F32, tag="eps", name="eps")
        nc.vector.memset(eps_t[:], LN_EPS)
        ident = const.tile([P, P], BF16, tag="ident", name="ident")
        make_identity(nc, ident[:])

        # weights: K-chunk tensors
        wq_s = [const.tile([P, 4 * P], BF16, tag=f"wq{k}", name=f"wq{k}") for k in range(2)]
        wk_s = [const.tile([P, 4 * P], BF16, tag=f"wk{k}", name=f"wk{k}") for k in range(2)]
        wvm_s = [const.tile([P, H * IN_C], BF16, tag=f"wvm{k}", name=f"wvm{k}") for k in range(2)]
        w1_s = [const.tile([P, EXPAND], BF16, tag=f"w1{k}", name=f"w1{k}") for k in range(2)]
        for k in range(2):
            nc.sync.dma_start(out=wq_s[k][:], in_=wq_e[k * P:(k + 1) * P, :])
            nc.sync.dma_start(out=wk_s[k][:], in_=wk_e[k * P:(k + 1) * P, :])
            nc.sync.dma_start(out=wvm_s[k][:], in_=wvm_e[k * P:(k + 1) * P, :])
            nc.sync.dma_start(out=w1_s[k][:], in_=w1_e[k * P:(k + 1) * P, :])
        w2_s = const.tile([P, EXPAND // P * IN_C], BF16, tag="w2", name="w2")  # [128, 8*256]
        nc.sync.dma_start(
            out=w2_s[:].rearrange("p (e c) -> p e c", e=EXPAND // P),
            in_=w2_e.ap().rearrange("(e p) c -> p e c", p=P),
        )

        # ---- persistent activations ----
        ln_b_T = [persist.tile([P, N], BF16, tag=f"lnbT{k}", name=f"lnbT{k}") for k in range(2)]
        qT = [persist.tile([P, NQ], BF16, tag=f"qT{g}", name=f"qT{g}") for g in range(4)]
        kT = [persist.tile([P, N], BF16, tag=f"kT{g}", name=f"kT{g}") for g in range(4)]
        x2acc = [persist.tile([P, IN_C], F32, tag=f"x2acc{i}", name=f"x2acc{i}") for i in range(NT_Q)]
        x2T = [persist.tile([P, NQ], BF16, tag=f"x2T{k}", name=f"x2T{k}") for k in range(2)]
        hT = persist.tile([P, EXPAND // P * NQ], BF16, tag="hT", name="hT")  # [128, 8*1024]
        ve = [persist.tile([P, NT_B * VE_W], BF16, tag=f"ve{i}", name=f"ve{i}")
              for i in range(2)]
        # ones columns at 256 + 257k, constant for the whole kernel
        for i in range(2):
            nc.gpsimd.memset(ve[i][:, IN_C::IN_C + 1], 1.0)

        # ---- LayerNorm: stats on Vector, sqrt + normalize on Scalar
        def ln_tile_sb(x_ap):
            stats = stats_p.tile([P, 6], F32, tag="stats", name="stats")
            mv = stats_p.tile([P, 2], F32, tag="mv", name="mv")
            nc.vector.bn_stats(out=stats[:], in_=x_ap)
            nc.vector.bn_aggr(out=mv[:], in_=stats[:])
            sd = stats_p.tile([P, 1], F32, tag="sd", name="sd")
            nc.scalar.activation(out=sd[:], in_=mv[:, 1:2], func=AF.Sqrt,
                                 bias=eps_t[:])
            nc.vector.reciprocal(out=sd[:], in_=sd[:])
            nmsd = stats_p.tile([P, 1], F32, tag="nmsd", name="nmsd")
            nc.vector.tensor_scalar(
                out=nmsd[:], in0=mv[:, 0:1], scalar1=sd[:], scalar2=-1.0,
                op0=ALU.mult, op1=ALU.mult)
            xnb = lnt.tile([P, IN_C], BF16, tag="ln_xnb", name="ln_xnb")
            nc.scalar.activation(out=xnb[:], in_=x_ap, func=AF.Identity,
                                 scale=sd[:], bias=nmsd[:])
            return xnb

        def transpose_into(dstlist, col, xnb):
            for k in range(2):
                tp = mm_ps.tile([P, P], BF16, tag="proj", name="proj")
                nc.tensor.transpose(tp[:], xnb[:, k * P:(k + 1) * P], ident[:])
                nc.vector.tensor_copy(out=dstlist[k][:, col:col + P], in_=tp[:])

        # ---- Phase A: LN1 over the 16 batch tiles (queries are tiles 0-7)
        for t in range(NT_B):
            xnb = ln_tile_sb(xbig[:, t * IN_C:(t + 1) * IN_C])
            transpose_into(ln_b_T, t * P, xnb)

        # ---- init x2acc = xq + battn (on GpSimd) ----
        for qi in range(NT_Q):
            nc.gpsimd.tensor_add(out=x2acc[qi][:],
                                 in0=xbig[:, qi * IN_C:(qi + 1) * IN_C],
                                 in1=battn_r[:])

        # ---- Phase B: q/k projections (channel-major, 2 heads/block);
        # k before q within a block so S(U0) can start as early as possible
        for g in range(4):
            for nn in range(N // 512):
                ps = mm_ps.tile([P, 512], F32, tag="proj", name="proj")
                for k in range(2):
                    nc.tensor.matmul(
                        ps[:], wk_s[k][:, g * P:(g + 1) * P],
                        ln_b_T[k][:, nn * 512:(nn + 1) * 512],
                        start=(k == 0), stop=(k == 1))
                nc.vector.tensor_scalar_add(
                    out=kT[g][:, nn * 512:(nn + 1) * 512], in0=ps[:],
                    scalar1=bk_t[:, g:g + 1])
            for nn in range(NQ // 512):
                ps = mm_ps.tile([P, 512], F32, tag="proj", name="proj")
                for k in range(2):
                    nc.tensor.matmul(
                        ps[:], wq_s[k][:, g * P:(g + 1) * P],
                        ln_b_T[k][:, nn * 512:(nn + 1) * 512],
                        start=(k == 0), stop=(k == 1))
                nc.vector.tensor_scalar_add(
                    out=qT[g][:, nn * 512:(nn + 1) * 512], in0=ps[:],
                    scalar1=bq_t[:, g:g + 1])

        # ---- Phase C: attention, software-pipelined over 8 (pair, qb) units.
        # Unit i runs its S matmuls (2-band concurrent: the pair's heads sit
        # at different 32-partition bands of the PE array) + exp, exp-paced on
        # ScalarE; unit i-1's PV chunks and later pairs' ve-builds fill the
        # TensorE stalls in between.
        units = [(p, qb) for p in range(4) for qb in range(2)]

        def ve_chunk(p, t):
            # values for heads 2p, 2p+1, key-tile t: [token, 514] with ones
            ps = mm_ps.tile([P, 512], F32, tag="proj", name="proj")
            for k in range(2):
                nc.tensor.matmul(
                    ps[:], ln_b_T[k][:, t * P:(t + 1) * P],
                    wvm_s[k][:, 2 * p * IN_C:(2 * p + 2) * IN_C],
                    start=(k == 0), stop=(k == 1))
            dst = ve[p % 2][:, t * VE_W:(t + 1) * VE_W] \
                .rearrange("p (h c) -> p h c", h=2)[:, :, 0:IN_C]
            src = ps[:].rearrange("p (h c) -> p h c", h=2)
            nc.vector.tensor_copy(out=dst, in_=src)

        # step (i, kc2) -> ve chunks to interleave, scheduled so each pair's
        # ve is complete before its first PV drain and never overwrites a ve
        # buffer still being read by in-flight PV units.
        VE_SCHED = {0: [(0, 2)], 1: [(1, 2)], 3: [(2, 1)],
                    4: [(2, 1, 8)], 5: [(3, 1)], 6: [(3, 1, 8)]}

        def s_group(u, ex, kc2):
            (p, qb) = u
            sts = [st_ps.tile([P, 1024], F32, tag="st", name="st")
                   for _ in range(2)]
            for j in range(2):
                kc = kc2 * 2 + j
                for h2 in range(2):
                    b = h2 * 64
                    nc.tensor.matmul(
                        sts[h2][:, j * 512:(j + 1) * 512],
                        kT[p][b:b + 32, kc * P:(kc + 1) * P],
                        qT[p][b:b + 32, qb * 512:(qb + 1) * 512],
                        start=True, stop=True)
            for h2 in range(2):
                nc.scalar.activation(
                    out=ex[h2][:, kc2 * 1024:(kc2 + 1) * 1024],
                    in_=sts[h2][:], func=AF.Exp)

        def pv_chunk(u, ex, c):
            (p, qb) = u
            h2, qt = c // 4, c % 4
            op = o_ps.tile([P, IN_C + 1], F32, tag="opv", name="opv")
            for kc in range(NT_B):
                nc.tensor.matmul(
                    op[:],
                    ex[h2][:, kc * 512 + qt * P: kc * 512 + (qt + 1) * P],
                    ve[p % 2][:, kc * VE_W + h2 * (IN_C + 1):
                              kc * VE_W + h2 * (IN_C + 1) + IN_C + 1],
                    start=(kc == 0), stop=(kc == NT_B - 1))
            rc = stats_p.tile([P, 1], F32, tag="rc", name="rc")
            nc.vector.reciprocal(out=rc[:], in_=op[:, IN_C:IN_C + 1])
            qi = qb * 4 + qt
            nc.vector.scalar_tensor_tensor(
                out=x2acc[qi][:], in0=op[:, 0:IN_C], scalar=rc[:],
                in1=x2acc[qi][:], op0=ALU.mult, op1=ALU.add)

        # ---- Phase D building blocks (interleaved into the final PV drain)
        def ln2_unit(qi):
            xnb = ln_tile_sb(x2acc[qi][:])
            transpose_into(x2T, qi * P, xnb)
            # after LN2 read, fold b2 into the residual
            nc.vector.tensor_add(out=x2acc[qi][:], in0=x2acc[qi][:],
                                 in1=b2_r[:])

        def ffn1_unit(nh, ebs):
            for eb in ebs:
                ps = mm_ps.tile([P, 512], F32, tag="proj", name="proj")
                for k in range(2):
                    nc.tensor.matmul(
                        ps[:], w1_s[k][:, eb * P:(eb + 1) * P],
                        x2T[k][:, nh * 512:(nh + 1) * 512],
                        start=(k == 0), stop=(k == 1))
                nc.scalar.activation(
                    out=hT[:, eb * NQ + nh * 512: eb * NQ + (nh + 1) * 512],
                    in_=ps[:], func=AF.Gelu, bias=b1_t[:, eb:eb + 1])

        def ffn2_unit(qi):
            op = o_ps.tile([P, IN_C + 1], F32, tag="opv", name="opv")
            for eb in range(EXPAND // P):
                nc.tensor.matmul(
                    op[:, 0:IN_C],
                    hT[:, eb * NQ + qi * P: eb * NQ + (qi + 1) * P],
                    w2_s[:, eb * IN_C:(eb + 1) * IN_C],
                    start=(eb == 0), stop=(eb == EXPAND // P - 1))
            ot = on_p.tile([P, IN_C], F32, tag="out", name="out")
            nc.vector.tensor_add(out=ot[:], in0=op[:, 0:IN_C], in1=x2acc[qi][:])
            nc.sync.dma_start(out=out_es[qi][:], in_=ot[:])

        def ln2a(qi):
            xnb = ln_tile_sb(x2acc[qi][:])
            transpose_into(x2T, qi * P, xnb)

        # FFN work for query tiles 0-3 (final once unit 6 drained) runs
        # inside unit 7's PV drain; step index -> thunks.
        def _b2adds03():
            for qi in range(4):
                nc.vector.tensor_add(out=x2acc[qi][:], in0=x2acc[qi][:],
                                     in1=b2_r[:])
        FFN_A = {
            0: [lambda: ln2a(0)], 1: [lambda: ln2a(1)],
            2: [lambda: ln2a(2)], 3: [lambda: ln2a(3), _b2adds03],
            4: [lambda: ffn1_unit(0, range(0, 4))],
            5: [lambda: ffn1_unit(0, range(4, 8))],
            6: [lambda: ffn2_unit(0), lambda: ffn2_unit(1)],
            7: [lambda: ffn2_unit(2), lambda: ffn2_unit(3)],
        }

        exps_of = {}
        for i in range(len(units) + 1):
            if i < len(units):
                exps_of[i] = [
                    exps_p.tile([P, NT_B * 512], BF16, tag="exps", name="exps")
                    for _ in range(2)]
            for kc2 in range(8):
                if i < len(units):
                    s_group(units[i], exps_of[i], kc2)
                for sched in VE_SCHED.get(i, []):
                    p_, n_ = sched[0], sched[1]
                    t0 = (sched[2] if len(sched) > 2 else 0) + kc2 * n_
                    for t in range(t0, t0 + n_):
                        ve_chunk(p_, t)
                if i > 0:
                    pv_chunk(units[i - 1], exps_of[i - 1], kc2)
            if i > 0:
                del exps_of[i - 1]

        # ---- Phase D ----
        for qi in range(NT_Q):
            ln2a(qi)
        for qi in range(NT_Q):
            nc.vector.tensor_add(out=x2acc[qi][:], in0=x2acc[qi][:],
                                 in1=b2_r[:])
        for nh in range(NQ // 512):
            ffn1_unit(nh, range(EXPAND // P))
            for qi in range(nh * 4, nh * 4 + 4):
                ffn2_unit(qi)

    if do_compile:
        nc.compile()
    return nc


def _to_bf16(a):
    return np.asarray(a, dtype=np.float32).astype(ml_dtypes.bfloat16)


def kernel(x, ln1_g, ln1_b, Wqkv, bqkv, Wm, bm, ln2_g, ln2_b, W1, b1, W2, b2):
    global _NC
    x = np.asarray(x, dtype=np.float32)
    ln1_g = np.asarray(ln1_g, dtype=np.float32)
    ln1_b = np.asarray(ln1_b, dtype=np.float32)
    Wqkv = np.asarray(Wqkv, dtype=np.float32)
    bqkv = np.asarray(bqkv, dtype=np.float32)
    Wm = np.asarray(Wm, dtype=np.float32)
    bm = np.asarray(bm, dtype=np.float32)
    ln2_g = np.asarray(ln2_g, dtype=np.float32)
    ln2_b = np.asarray(ln2_b, dtype=np.float32)
    W1 = np.asarray(W1, dtype=np.float32)
    b1 = np.asarray(b1, dtype=np.float32)
    W2 = np.asarray(W2, dtype=np.float32)
    b2 = np.asarray(b2, dtype=np.float32)

    scale = HEAD_C ** -0.5
    # fold LN1 gain into the q/k/v weight rows; LN1 bias into the biases
    Wq_f = ln1_g[:, None] * Wqkv[:, :IN_C]
    Wk_f = ln1_g[:, None] * Wqkv[:, IN_C:2 * IN_C]
    Wv_f = ln1_g[:, None] * Wqkv[:, 2 * IN_C:]
    bq_f = bqkv[:IN_C] + ln1_b @ Wqkv[:, :IN_C]
    bk_f = bqkv[IN_C:2 * IN_C] + ln1_b @ Wqkv[:, IN_C:2 * IN_C]
    bv_f = bqkv[2 * IN_C:] + ln1_b @ Wqkv[:, 2 * IN_C:]

    # head-padded q/k layouts: block b = heads 2b (col 0) / 2b+1 (col 64)
    wq = np.zeros((IN_C, 4 * P), np.float32)
    wk = np.zeros((IN_C, 4 * P), np.float32)
    bq = np.zeros((P, 4), np.float32)
    bk = np.zeros((P, 4), np.float32)
    for h in range(H):
        g, off = h // 2, 64 * (h % 2)
        wq[:, g * P + off: g * P + off + HEAD_C] = \
            Wq_f[:, h * HEAD_C:(h + 1) * HEAD_C] * scale
        wk[:, g * P + off: g * P + off + HEAD_C] = \
            Wk_f[:, h * HEAD_C:(h + 1) * HEAD_C]
        bq[off:off + HEAD_C, g] = bq_f[h * HEAD_C:(h + 1) * HEAD_C] * scale
        bk[off:off + HEAD_C, g] = bk_f[h * HEAD_C:(h + 1) * HEAD_C]
    wq = _to_bf16(wq)
    wk = _to_bf16(wk)
    wvm = np.empty((IN_C, H * IN_C), np.float32)
    for h in range(H):
        Wv_h = Wv_f[:, h * IN_C:(h + 1) * IN_C]
        Wm_h = Wm[h * IN_C:(h + 1) * IN_C, :]
        wvm[:, h * IN_C:(h + 1) * IN_C] = Wv_h @ Wm_h
    wvm = _to_bf16(wvm)
    battn = (bm + bv_f @ Wm).astype(np.float32)

    # fold LN2 gain into W1 rows, LN2 bias into b1
    W1_f = ln2_g[:, None] * W1
    b1_f = b1 + ln2_b @ W1
    b1_l = np.ascontiguousarray(b1_f.reshape(EXPAND // P, P).T).astype(np.float32)

    common = dict(
        wq=wq, wk=wk, wvm=wvm, w1=_to_bf16(W1_f), w2=_to_bf16(W2),
        battn=battn, b2=b2, bq=bq, bk=bk, b1=b1_l,
    )
    in_maps = []
    for c in range(8):
        b, qh = c // 2, c % 2
        # rotate the batch so this core's query rows are rows 0..1023
        xb = x[b] if qh == 0 else np.concatenate([x[b, NQ:], x[b, :NQ]])
        in_maps.append(dict(xb=np.ascontiguousarray(xb), **common))

    global LAST_RESULT
    if _NC is None:
        _NC = _build()
    res = run_bass_kernel_spmd(_NC, in_maps, core_ids=list(range(8)))
    LAST_RESULT = res  # exposes exec_time_ns when BASS_TRACE=1 is set
    out = np.concatenate(
        [np.asarray(res.results[c][f"out{i}"])
         for c in range(8) for i in range(NT_Q)], axis=0)
    return out.reshape(B, N, IN_C)


LAST_RESULT = None
